# revision 1
# baseline (speedup 1.0000x reference)
"""Correlation layer (FlowNet-style) on 8 Trainium2 NeuronCores.

Strategy (data-parallel over batch, one batch element per core):
  out[d,h,w] = sum_c x1[c,h,w] * x2p[c, h+di+4, w+dj+4] / sqrt(C),
  di,dj in [-4,4], 80 displacements (81 minus center).

Per core, banded-Gram matmuls with displacement packing:
  - lhsT = x1 rows [24g-8+rho'' : +32) x 4 w-cols  -> M = 32*4 = 128
    (m = rho''*4 + ww, rho'' = rr - di + 4)
  - rhs  = x2p rows [24g : 24g+24) x 12 padded-w cols -> N = 24*12 = 288
    (n = rr*12 + u, u = ww + dj + 4)
  - psum[m, n] is useful iff rho'' = rr - di + 4 and u - ww in [0, 9).
    For fixed rr all useful elements live in partitions [4rr, 4rr+36),
    so a pure-partition-step DMA can ship a 75%-dense window per rr.

Pipeline: DMA x2p slab (24 rows) -> 288-col f32r/bf16/f32 matmuls ->
DVE/ACT copy PSUM->SBUF staging (relayout to (rr, wb, u)) -> per-rr
window DMA (576B contiguous runs) -> DRAM; host decodes windows into
the [80, H, W] layout with pure slicing.
"""

import math
import numpy as np
from contextlib import ExitStack

B, C, H, W = 8, 128, 128, 192
MD = 4
NDISP = 81

R = 24          # x2p rows per group
NG = 6          # row groups (covers 144 padded rows)
WW = 4          # output w-cols per block
NWB = W // WW   # 48 blocks
CHUNK = 12      # blocks per staging chunk
NCH = NWB // CHUNK  # 4
HP = 152        # x1pad rows: 8 zero + 128 + 16 zero
X1B = NWB * 32 * WW        # per-group x1 block slab: 6144 elems/partition
X2R, X2C = NG * R, W + 8   # 144 x 200
UB = WW + 8     # 12 rhs cols per block
NMM = R * UB    # 288 matmul free size
ROWSZ = R * CHUNK * UB     # staging free size 3456
WIN = 36                   # band window partitions per rr
OUTSZ = NG * NCH * R * WIN * (CHUNK * UB)  # per-core band elements

MM_DTYPE = "float32"      # "float32" | "float32r" | "bfloat16"
EVAC_PATTERN = "vvs"       # per-copy engine cycle: v=vector, s=scalar

_CACHE = {}


def _build(mm_dtype, evac_pattern):
    import concourse.bass as bass
    import concourse.tile as tile
    from concourse import bacc, mybir

    in_dt = mybir.dt.bfloat16 if mm_dtype == "bfloat16" else mybir.dt.float32
    f32 = mybir.dt.float32

    nc = bacc.Bacc("TRN2", target_bir_lowering=False, debug=False, num_devices=8)
    x1d = nc.dram_tensor("x1p", [128, NG * X1B], in_dt, kind="ExternalInput")
    x2d = nc.dram_tensor("x2p", [128, X2R * X2C], in_dt, kind="ExternalInput")
    outd = nc.dram_tensor("band", [OUTSZ], f32, kind="ExternalOutput")

    with tile.TileContext(nc) as tc, ExitStack() as ctx:
        x1pool = ctx.enter_context(tc.tile_pool(name="x1", bufs=2))
        x2pool = ctx.enter_context(tc.tile_pool(name="x2", bufs=2))
        pspool = ctx.enter_context(tc.tile_pool(name="ps", bufs=4, space="PSUM"))
        stpool = ctx.enter_context(tc.tile_pool(name="st", bufs=3))

        ev = 0
        for g in range(NG):
            x1t = x1pool.tile([128, X1B], in_dt, tag="x1")
            nc.sync.dma_start(x1t[:], x1d.ap()[:, g * X1B:(g + 1) * X1B])
            x2t = x2pool.tile([128, R * X2C], in_dt, tag="x2")
            nc.sync.dma_start(x2t[:], x2d.ap()[:, g * R * X2C:(g + 1) * R * X2C])
            x2v = x2t[:].rearrange("p (r u) -> p r u", r=R)
            for chn in range(NCH):
                stt = stpool.tile([128, ROWSZ], f32, tag="st")
                stv = stt[:].rearrange("p (r b u) -> p r b u", r=R, b=CHUNK)
                for wp in range(CHUNK // 2):
                    pst = pspool.tile([128, 2, 512], f32, tag="ps")
                    for k in range(2):
                        wb = chn * CHUNK + wp * 2 + k
                        lhsT = x1t[:, wb * 128:(wb + 1) * 128]
                        rhs = x2v[:, :, wb * WW:wb * WW + UB]
                        if mm_dtype == "float32r":
                            lhsT = lhsT.bitcast(mybir.dt.float32r)
                            rhs = rhs.bitcast(mybir.dt.float32r)
                        nc.tensor.matmul(pst[:, k, 0:NMM], lhsT, rhs,
                                         start=True, stop=True)
                    # evac pair -> staging (rr, wb_local, u), (k, rr, u)->(rr, k, u)
                    src = pst[:, :, 0:NMM].rearrange(
                        "p a (r u) -> p r a u", r=R).copy()
                    dst = stv[:, :, wp * 2:wp * 2 + 2, :]
                    if evac_pattern[ev % len(evac_pattern)] == "v":
                        nc.vector.tensor_copy(dst, src)
                    else:
                        nc.scalar.copy(dst, src)
                    ev += 1
                # band window DMAs: one per rr
                for rr in range(R):
                    src = bass.AP(stt[:].tensor,
                                  (4 * rr) * ROWSZ + rr * (CHUNK * UB),
                                  [[ROWSZ, WIN], [1, CHUNK * UB]])
                    dsto = ((g * NCH + chn) * R + rr) * WIN * (CHUNK * UB)
                    dst = bass.AP(outd.ap().tensor, dsto,
                                  [[CHUNK * UB, WIN], [1, CHUNK * UB]])
                    nc.sync.dma_start(dst, src)

    nc.compile()
    return nc


def _get_nc():
    key = (MM_DTYPE, EVAC_PATTERN)
    if key not in _CACHE:
        _CACHE[key] = _build(*key)
    return _CACHE[key]


def _prep_inputs(x1, x2):
    import ml_dtypes
    np_dt = ml_dtypes.bfloat16 if MM_DTYPE == "bfloat16" else np.float32
    in_maps = []
    for b in range(B):
        x1p = np.zeros((128, HP, NWB, WW), np_dt)
        x1p.reshape(128, HP, W)[:, 8:8 + H, :] = np.asarray(x1[b], np.float32)
        win = np.stack([x1p[:, R * g:R * g + 32] for g in range(NG)], axis=1)
        x1b = win.transpose(0, 1, 3, 2, 4).reshape(128, NG * X1B)
        x2p = np.zeros((128, X2R, X2C), np_dt)
        x2p[:, 4:4 + H, 4:4 + W] = np.asarray(x2[b], np.float32)
        in_maps.append({"x1p": np.ascontiguousarray(x1b),
                        "x2p": x2p.reshape(128, X2R * X2C)})
    return in_maps


def _decode(band, out81):
    """band: per-core [OUTSZ] f32 -> out81 [81, H, W] (scaled later)."""
    arr = band.reshape(NG, NCH, R, 9, 4, CHUNK, UB)  # (g,c,rr,t,ww,wb,u)
    for ww in range(WW):
        sub = arr[:, :, :, :, ww, :, ww:ww + 9]       # (g,c,rr,t,wb,dj)
        tmat = sub.transpose(3, 5, 0, 2, 1, 4).reshape(9, 9, NG * R, NCH * CHUNK)
        for t in range(9):
            di_idx = 8 - t                            # di = 4 - t
            r2lo = di_idx                             # r2 = h + di + 4
            out81[di_idx * 9:di_idx * 9 + 9, :, ww::WW] = \
                tmat[t, :, r2lo:r2lo + H, :]
    return out81


def kernel(x1, x2):
    from concourse.bass_utils import run_bass_kernel_spmd

    x1 = np.asarray(x1, np.float32)
    x2 = np.asarray(x2, np.float32)
    nc = _get_nc()
    in_maps = _prep_inputs(x1, x2)
    res = run_bass_kernel_spmd(nc, in_maps, core_ids=list(range(8)))

    inv_sqrt_c = np.float32(1.0 / math.sqrt(C))
    out = np.empty((B, NDISP - 1, H, W), np.float32)
    out81 = np.empty((NDISP, H, W), np.float32)
    for b in range(B):
        _decode(res.results[b]["band"], out81)
        out[b] = np.delete(out81, 40, axis=0) * inv_sqrt_c
    return out



# revision 2
# speedup vs baseline: 893.0494x; 893.0494x over previous
"""Correlation layer (FlowNet-style) on 8 Trainium2 NeuronCores.

Data-parallel over batch (one element per core). Per core, banded-Gram
matmuls with displacement packing:
  - lhsT = x1 rows [24g-8+rho'' : +32) x 4 w-cols -> M = 32*4 = 128
    (host-blocked: the matmul stationary operand must be one contiguous
    free dim per the BIR verifier)
  - rhs  = x2p rows [24g : 24g+24) x 12 padded-w cols -> N = 24*12 = 288
  - psum[m, n] useful iff rho'' = rr - di + 4 and u - ww in [0, 9).

Perf design (validated with CoreSim cost model + HW repeat-slope timing;
the original version was 99% bound on the SP engine issuing 576 tiny
per-rr window DMAs at ~600ns each, 392us simulated / ~508us HW):
  - bf16 inputs + bf16 matmuls: 1 PE cycle/row instead of 4 (f32), half
    the input DMA traffic. PSUM accumulates f32. bf16 band output.
    (rel err ~4e-3 vs the 2e-2 gate)
  - Band-window DMAs merged 2 rr per descriptor set (pure strides only;
    mixed-stride APs break the tile dep tracker), issue alternating
    SP / Pool(SWDGE); PSUM evac copies alternate DVE / ACT (Pool cannot
    read PSUM).
  - Software-pipelined input prefetch (depth 3) + triple-buffered
    staging so group g+1 compute overlaps group g band-window DMAs.
Simulated 85us/exec; measured ~54us/exec steady-state on HW (repeat
slope), ~9.4x the original kernel. Memory-roofline bound: ~22MB HBM
traffic/core/exec.
"""

import math
import numpy as np
from contextlib import ExitStack

B, C, H, W = 8, 128, 128, 192
MD = 4
NDISP = 81

R = 24            # x2p rows per group
NG = 6            # row groups
WW = 4            # output w-cols per block
NWB = W // WW     # 48 blocks
UB = WW + 8       # 12 rhs cols per block
NMM = R * UB      # 288 matmul free size
HP = 152          # x1 padded rows: 8 zero + 128 + 16 zero
X1SL = 32 * W     # per-group x1 slab elems/partition
X2C = W + 8       # 200
RS = R * NWB * UB          # 13824 staging elems/partition
WIN = 36                   # band window partitions per rr
WINR = 2                   # rr values merged per band-window DMA
NQ = R // WINR             # window DMAs per group
WQP = WIN + 4 * (WINR - 1)     # partitions per merged window
WQE = WINR * NWB * UB          # free elems per merged window
GRP_OUT = NQ * WQP * WQE       # 663552 at WINR=4
OUTSZ = NG * GRP_OUT

IN_DTYPE = "bfloat16"     # "bfloat16" | "float32"
MM_DTYPE = "bfloat16"     # "bfloat16" | "float32" | "float32r" (must match IN unless f32->f32r)
BAND_DTYPE = "bfloat16"   # "float32" | "bfloat16"
# x1 layout: host-blocked per-group slabs [(wb, rho'', ww)] — the matmul
# stationary operand must be a single contiguous free dim (BIR verifier:
# "RHS AP can only have one free dimension"), so unblocked x1 is illegal.
X1_MODE = "slab"
EVAC = "vs"               # per-copy engine cycle: v=DVE, s=ACT (Pool can't read PSUM)
WDMA = "yg"               # window-DMA issue engine cycle: y=SP, s=ACT, g=Pool(SWDGE)
PREFETCH = 3              # input-DMA software pipeline depth (<= input pool bufs)
IN_BUFS = 3
ST_BUFS = 3               # staging pool depth

_CACHE = {}


def _build(in_dtype_s, mm_dtype_s, band_dtype_s, evac, wdma, x1_mode, nrep=1):
    import concourse.bass as bass
    import concourse.tile as tile
    from concourse import bacc, mybir

    dtmap = {"bfloat16": mybir.dt.bfloat16, "float32": mybir.dt.float32,
             "float32r": mybir.dt.float32r}
    in_dt = dtmap[in_dtype_s]
    band_dt = dtmap[band_dtype_s]
    f32 = mybir.dt.float32

    nc = bacc.Bacc("TRN2", target_bir_lowering=False, debug=False, num_devices=8)
    if nrep > 1:
        # the NEFF cache hashes HLO structure only (not the embedded BIR);
        # an nrep-dependent input shape keeps repeat variants distinct
        nc.dram_tensor("reptag", [1, 8 * nrep], mybir.dt.float32,
                       kind="ExternalInput")
    x1d = nc.dram_tensor("x1p", [128, NG * X1SL], in_dt, kind="ExternalInput")
    x2d = nc.dram_tensor("x2p", [128, NG * R * X2C], in_dt, kind="ExternalInput")
    outd = nc.dram_tensor("band", [OUTSZ], band_dt, kind="ExternalOutput")

    with tile.TileContext(nc) as tc, ExitStack() as ctx:
        x1pool = ctx.enter_context(tc.tile_pool(name="x1", bufs=IN_BUFS))
        x2pool = ctx.enter_context(tc.tile_pool(name="x2", bufs=IN_BUFS))
        pspool = ctx.enter_context(tc.tile_pool(name="ps", bufs=4, space="PSUM"))
        stpool = ctx.enter_context(tc.tile_pool(name="st", bufs=ST_BUFS))

        ev = 0
        wd = 0
        x1ts, x2ts = {}, {}

        def prefetch(i):
            g = i % NG
            x1t = x1pool.tile([128, X1SL], in_dt, tag="x1")
            nc.sync.dma_start(x1t[:], x1d.ap()[:, g * X1SL:(g + 1) * X1SL])
            x1ts[i] = x1t
            x2t = x2pool.tile([128, R * X2C], in_dt, tag="x2")
            nc.sync.dma_start(x2t[:], x2d.ap()[:, g * R * X2C:(g + 1) * R * X2C])
            x2ts[i] = x2t

        for i in range(PREFETCH):
            prefetch(i)
        for i in range(NG * nrep):
            if i + PREFETCH < NG * nrep:
                prefetch(i + PREFETCH)
            g = i % NG
            x1t, x2t = x1ts.pop(i), x2ts.pop(i)
            x2v = x2t[:].rearrange("p (r u) -> p r u", r=R)
            stt = stpool.tile([128, RS], band_dt, tag="st")
            stv = stt[:].rearrange("p (r b u) -> p r b u", r=R, b=NWB)
            for wp in range(NWB // 2):
                pst = pspool.tile([128, 2, 512], f32, tag="ps")
                for k in range(2):
                    wb = wp * 2 + k
                    lhsT = x1t[:, wb * 128:(wb + 1) * 128]
                    rhs = x2v[:, :, wb * WW:wb * WW + UB]
                    if mm_dtype_s == "float32r":
                        lhsT = lhsT.bitcast(mybir.dt.float32r)
                        rhs = rhs.bitcast(mybir.dt.float32r)
                    nc.tensor.matmul(pst[:, k, 0:NMM], lhsT, rhs,
                                     start=True, stop=True)
                src = pst[:, :, 0:NMM].rearrange("p a (r u) -> p r a u", r=R).copy()
                dst = stv[:, :, wp * 2:wp * 2 + 2, :]
                e = evac[ev % len(evac)]
                ev += 1
                if e == "v":
                    nc.vector.tensor_copy(dst, src)
                elif e == "s":
                    nc.scalar.copy(dst, src)
                else:
                    nc.gpsimd.tensor_copy(dst, src)
            # band window DMAs: WINR consecutive rr merged per DMA (pure
            # strides only — mixed-stride APs break the tile dep tracker),
            # issue rotated across engines
            rowlen = NWB * UB
            for q in range(NQ):
                rr0 = q * WINR
                src = bass.AP(stt[:].tensor, (4 * rr0) * RS + rr0 * rowlen,
                              [[RS, WQP], [1, WQE]])
                dst = bass.AP(outd.ap().tensor,
                              g * GRP_OUT + q * WQP * WQE,
                              [[WQE, WQP], [1, WQE]])
                e = wdma[wd % len(wdma)]
                wd += 1
                if e == "y":
                    nc.sync.dma_start(dst, src)
                elif e == "s":
                    nc.scalar.dma_start(dst, src)
                else:
                    nc.gpsimd.dma_start(dst, src)

    nc.compile()
    return nc


def _get_nc():
    key = (IN_DTYPE, MM_DTYPE, BAND_DTYPE, EVAC, WDMA, X1_MODE)
    if key not in _CACHE:
        _CACHE[key] = _build(*key)
    return _CACHE[key]


def _prep_inputs(x1, x2):
    import ml_dtypes
    np_dt = ml_dtypes.bfloat16 if IN_DTYPE == "bfloat16" else np.float32
    in_maps = []
    for b in range(x1.shape[0]):
        x1p = np.zeros((128, HP, NWB, WW), np_dt)
        x1p.reshape(128, HP, W)[:, 8:8 + H, :] = x1[b]
        win = np.stack([x1p[:, 24 * g:24 * g + 32] for g in range(NG)], axis=1)
        slabs = win.transpose(0, 1, 3, 2, 4).reshape(128, NG * X1SL)
        x2p = np.zeros((128, NG * R, X2C), np_dt)
        x2p[:, 4:4 + H, 4:4 + W] = x2[b]
        in_maps.append({"x1p": np.ascontiguousarray(slabs),
                        "x2p": np.ascontiguousarray(x2p.reshape(128, NG * R * X2C))})
    return in_maps


def _decode(band, out81):
    """band: per-core [OUTSZ] -> out81 [81, H, W] (scaled later)."""
    raw = np.asarray(band, np.float32).reshape(NG, NQ, WQP // 4, WW, WINR, NWB, UB)
    if WINR == 1:
        arr = raw.reshape(NG, R, 9, WW, NWB, UB)
    else:
        bv = np.empty((NG, NQ, WINR, 9, WW, NWB, UB), np.float32)
        for b in range(WINR):
            bv[:, :, b] = raw[:, :, b:b + 9, :, b]
        arr = bv.reshape(NG, R, 9, WW, NWB, UB)
    for ww in range(WW):
        sub = arr[:, :, :, ww, :, ww:ww + 9]          # (g, rr, t, wb, dj)
        tmat = sub.transpose(2, 4, 0, 1, 3).reshape(9, 9, NG * R, NWB)
        for t in range(9):
            di_idx = 8 - t                             # di = 4 - t
            r2lo = di_idx
            out81[di_idx * 9:di_idx * 9 + 9, :, ww::WW] = \
                tmat[t, :, r2lo:r2lo + H, :]
    return out81


def kernel(x1, x2):
    from concourse.bass_utils import run_bass_kernel_spmd

    x1 = np.asarray(x1, np.float32)
    x2 = np.asarray(x2, np.float32)
    nc = _get_nc()
    in_maps = _prep_inputs(x1, x2)
    res = run_bass_kernel_spmd(nc, in_maps, core_ids=list(range(8)))

    inv_sqrt_c = np.float32(1.0 / math.sqrt(C))
    out = np.empty((B, NDISP - 1, H, W), np.float32)
    out81 = np.empty((NDISP, H, W), np.float32)
    for b in range(B):
        _decode(res.results[b]["band"], out81)
        out[b] = np.delete(out81, 40, axis=0) * inv_sqrt_c
    return out
